# revision 3
# baseline (speedup 1.0000x reference)
"""Trainium2 Bass kernel for nn_GAT_1580547975275 (2-layer GAT, N=100k, E=1.6M).

Strategy (graph/data parallel over 8 NeuronCores, SPMD single program):
- Nodes are ranked by in-degree (host), dealt round-robin to the 8 cores so
  every core sees an identical per-chunk max-degree profile (one shared
  program).  Each core owns M=12500 destination nodes; incoming edges of a
  node occupy K slots of a [128 nodes x K] grid (K = per-chunk max degree).
- Layer-1 message linearity: sum_e alpha_e * h[src_e] = (sum_e alpha_e *
  x[src_e]) @ W1, so per edge we only gather x[src] (16B), not h (256B).
  Attention logits a_s[src] are likewise computed on-device from gathered x
  via folded weights U_s = einsum(W1, att_src1).
- Gathers use the fast SWDGE dma_gather with int16 indices.  Node payloads
  are quad-packed: table row r (256B stride) holds x of gid 4r..4r+3, so row
  indices fit int16 (25088 rows).  A host-built one-hot `sel` mask picks the
  right quarter of each gathered row (and zeroes padding slots).
- Softmax per destination runs over the K axis with an additive -1e9 mask on
  padding slots; the max-subtraction is dropped (mathematically identity).
- h2 (layer-2 scalar feature) is AllGathered across cores inside the same
  NEFF, written into spare columns of the quad table, and layer 2 repeats the
  same gather/softmax with a scalar payload.
"""

import os
import sys

for _p in ("/opt/trn_rl_repo", "/root/.axon_site/_ro/trn_rl_repo"):
    if os.path.isdir(_p) and _p not in sys.path:
        sys.path.insert(0, _p)

import numpy as np

import concourse.bacc as bacc
import concourse.bass as bass
import concourse.mybir as mybir
import concourse.tile as tile
from concourse import ap_utils, bass_utils
from concourse.bass import MemorySpace

# ---------------------------------------------------------------- constants
N = 100000
FIN = 4
HID = 8
HEADS = 8
NEG_SLOPE = 0.2

NCORES = 8
P = 128
M = N // NCORES            # 12500 nodes per core
T = (M + P - 1) // P       # 98 tiles per core
MPAD = T * P               # 12544
NPAD = NCORES * MPAD       # 100352
CT = 7                     # tiles per chunk
NCHUNK = T // CT           # 14
QROWS = NPAD // 4          # 25088 quad rows (int16-safe)
TBL_COLS = 64              # 256B row stride
GB = 4                     # gather blocks (x128 idx) per dma_gather (<=896 idx safe)
NEGBIG = -1.0e9

F32 = mybir.dt.float32
I16 = mybir.dt.int16


# ------------------------------------------------- relaxed dma_gather shim
def _dma_gather_small_elem(eng, out_ap, in_ap, idxs_ap, num_idxs, elem_size,
                           elem_step):
    """nc.gpsimd.dma_gather with the elem_size%256B assert relaxed.

    Vendored from concourse.bass.BassGpSimd.dma_gather (HBM-source,
    non-transpose path).  The 256B-multiple restriction belongs to the
    transpose mode; the ucode's non-transpose path takes elem_size and a
    256B-multiple row stride independently.
    """
    bassmod = sys.modules["concourse.bass"]
    assert idxs_ap.dtype == I16
    assert in_ap.dtype == out_ap.dtype
    elem_bytes = elem_size * mybir.dt.size(in_ap.dtype)
    assert elem_bytes > 0 and elem_bytes % 4 == 0
    assert in_ap.space == MemorySpace.DRAM
    assert idxs_ap.space == MemorySpace.SBUF
    assert out_ap.space == MemorySpace.SBUF
    assert ap_utils.ap_is_contiguous(in_ap.ap[1:])
    assert ap_utils.ap_is_contiguous(out_ap.ap[1:])
    assert ap_utils.ap_is_contiguous(idxs_ap.ap[1:])
    assert in_ap.ap[-1][1] == out_ap.ap[-1][1] == elem_size
    assert out_ap.ap[0][1] * out_ap.ap[1][1] == bassmod.round_up_to_multiple(
        num_idxs, 128)
    assert in_ap.ap[0][0] == elem_step
    stride_bytes = elem_step * mybir.dt.size(in_ap.dtype)
    stride_bytes_256 = bassmod.exact_div(stride_bytes, 256)
    assert stride_bytes_256 < 256

    _in_ap = eng.lower_ap_dma(in_ap, for_custom_bir_dma=True)
    _idxs_ap = eng.lower_ap(idxs_ap)
    _out_ap = eng.lower_ap(out_ap)
    return eng.add_instruction(
        mybir.InstDMAGatherAnt(
            name=eng.bass.get_next_instruction_name(),
            ins=[*_in_ap, _idxs_ap,
                 eng.lower_val_access(eng.to_reg(num_idxs))],
            outs=[_out_ap],
            transpose=False,
            num_idxs=num_idxs,
            elem_size=elem_size,
            stride_bytes_256=stride_bytes_256,
            gen_mode=0,
            single_packet=True,
            queue_num=0,
            sbuf_tokens_per_rank=0,
            sbuf_free_dim_per_rank=0,
            sbuf_free_dim_pad_per_rank=0,
            sbuf_byte_offset=0,
        ))


# ------------------------------------------------------------- host prep
def _prep(x, edge_index):
    src = np.concatenate([np.asarray(edge_index[0]),
                          np.arange(N, dtype=np.int64)])
    dst = np.concatenate([np.asarray(edge_index[1]),
                          np.arange(N, dtype=np.int64)])
    deg = np.bincount(dst, minlength=N)
    order = np.argsort(-deg, kind="stable")
    inv = np.empty(N, np.int64)
    inv[order] = np.arange(N)

    gid = (inv & 7) * MPAD + (inv >> 3)          # node -> gid

    deg_sorted = deg[order]
    Kq = np.empty(NCHUNK, np.int64)
    for q in range(NCHUNK):
        lo = q * CT * P * NCORES
        hi = min((q + 1) * CT * P * NCORES, N)
        Kq[q] = max(1, int(deg_sorted[lo:hi].max())) if lo < N else 1

    cols_q = CT * Kq                              # sel/emask columns per chunk
    coloff = np.concatenate([[0], np.cumsum(cols_q)])
    COLS = int(coloff[-1])                        # per-core grid columns
    TOT = COLS * P                                # per-core padded slots

    # per-edge slot position
    rd = inv[dst]
    cd = rd & 7
    md = rd >> 3
    td = md >> 7
    pd = md & 127
    qd = td // CT
    tq = td % CT
    s_ord = np.argsort(rd, kind="stable")
    rd_s = rd[s_ord]
    new = np.r_[True, rd_s[1:] != rd_s[:-1]]
    starts = np.where(new, np.arange(rd_s.size), 0)
    starts = np.maximum.accumulate(starts)
    k = np.empty(rd_s.size, np.int64)
    k[s_ord] = np.arange(rd_s.size) - starts

    col = coloff[qd] + tq * Kq[qd] + k            # grid column of each edge
    gs = gid[src]

    idxg = np.zeros((NCORES, P, COLS), np.int16)  # quad-row per slot (pad->0)
    sel = np.zeros((NCORES, P, COLS, 4), np.float32)
    emask = np.full((NCORES, P, COLS), NEGBIG, np.float32)
    idxg[cd, pd, col] = (gs >> 2).astype(np.int16)
    sel[cd, pd, col, gs & 3] = 1.0
    emask[cd, pd, col] = 0.0

    # pack idx lists: chunk list order i = (t*Kq+k)*128 + p  ->  [128, L/16]
    # with tile[pp, jj] = list[jj*16 + pp%16]
    idx_packed = np.empty((NCORES, P, TOT // 16), np.int16)
    po16 = 0
    for q in range(NCHUNK):
        L = int(cols_q[q]) * P
        blk = idxg[:, :, coloff[q]:coloff[q + 1]]       # [8, 128, CT*Kq]
        lst = blk.transpose(0, 2, 1)                    # [8, cols, 128] i-major
        lst = lst.reshape(NCORES, L // 16, 16)
        w = lst.transpose(0, 2, 1)                      # [8, 16, L/16]
        idx_packed[:, :, po16:po16 + L // 16] = np.tile(w, (1, 8, 1))
        po16 += L // 16

    # x quad table content (first 16 cols)
    xg16 = np.zeros((NPAD, FIN), np.float32)
    xg16[gid] = np.asarray(x, np.float32)
    xqx = xg16.reshape(QROWS, 16)

    # local x per core for a_d
    xl = np.empty((NCORES, P, T * FIN), np.float32)
    for c in range(NCORES):
        xl[c] = (xg16[c * MPAD:(c + 1) * MPAD]
                 .reshape(T, P, FIN).transpose(1, 0, 2).reshape(P, T * FIN))

    meta = dict(Kq=tuple(int(v) for v in Kq), COLS=COLS,
                coloff=tuple(int(v) for v in coloff))
    arrays = dict(idx_packed=idx_packed, sel=sel, emask=emask, xqx=xqx, xl=xl)
    return meta, arrays, order


def _fold_params(W1, att_src1, att_dst1, b1, W2, att_src2, att_dst2, b2):
    W1 = np.asarray(W1, np.float32)
    Wh = W1.reshape(FIN, HEADS, HID)                      # [f, h, c]
    us = np.einsum("fhc,hc->hf", Wh, np.asarray(att_src1, np.float32))
    ud = np.einsum("fhc,hc->hf", Wh, np.asarray(att_dst1, np.float32))
    v3 = Wh.transpose(1, 2, 0).reshape(1, HEADS * HID * FIN)  # [h, c, f]
    rep = lambda a: np.ascontiguousarray(np.tile(np.asarray(a, np.float32)
                                                 .reshape(1, -1), (128, 1)))
    return dict(
        us=rep(us), ud=rep(ud), v3=rep(v3),
        b1v=rep(b1), w2v=rep(W2),
        sw2=rep(np.asarray(W2, np.float32).sum()),
        as2=rep(att_src2), ad2=rep(att_dst2), b2v=rep(b2),
    )


# ---------------------------------------------------------- device program
def _build(meta):
    Kq = meta["Kq"]
    COLS = meta["COLS"]
    coloff = meta["coloff"]
    TOT16 = COLS * P // 16

    nc = bacc.Bacc("TRN2", target_bir_lowering=False, debug=False,
                   num_devices=NCORES)
    d_idx = nc.dram_tensor("idxg", [P, TOT16], I16, kind="ExternalInput")
    d_sel = nc.dram_tensor("sel", [P, COLS * 4], F32, kind="ExternalInput")
    d_em = nc.dram_tensor("emask", [P, COLS], F32, kind="ExternalInput")
    d_xqx = nc.dram_tensor("xqx", [QROWS, 16], F32, kind="ExternalInput")
    d_xl = nc.dram_tensor("xl", [P, T * FIN], F32, kind="ExternalInput")
    d_par = {k: nc.dram_tensor(k, [P, n], F32, kind="ExternalInput")
             for k, n in [("us", 32), ("ud", 32), ("v3", 256), ("b1v", 64),
                          ("w2v", 64), ("sw2", 1), ("as2", 1), ("ad2", 1),
                          ("b2v", 1)]}
    d_out = nc.dram_tensor("out", [P, T], F32, kind="ExternalOutput")

    AX = mybir.AxisListType.X
    OP = mybir.AluOpType
    ACT = mybir.ActivationFunctionType

    with tile.TileContext(nc) as tc:
        with tc.tile_pool(name="res", bufs=1) as res, \
             tc.tile_pool(name="io", bufs=2) as io, \
             tc.tile_pool(name="wk", bufs=2) as wk, \
             tc.tile_pool(name="dram", bufs=1, space="DRAM") as dram:

            # ---- resident small tensors
            c_par = {}
            for k, d in d_par.items():
                t = res.tile(list(d.shape), F32, tag=f"par_{k}")
                nc.sync.dma_start(out=t[:], in_=d[:])
                c_par[k] = t
            xl_t = res.tile([P, T * FIN], F32, tag="xl")
            nc.sync.dma_start(out=xl_t[:], in_=d_xl[:])

            # a_d_all[p, t, h] = sum_f xl[p,t,f] * ud[h,f]
            ad_all = res.tile([P, T * HEADS], F32, tag="ad_all")
            tmp_ad = res.tile([P, T * HEADS * FIN], F32, tag="tmp_ad")
            xl_r = xl_t[:].rearrange("p (t f) -> p t f", f=FIN)
            nc.vector.tensor_mul(
                out=tmp_ad[:].rearrange("p (t h f) -> p t h f", h=HEADS, f=FIN),
                in0=xl_r.unsqueeze(2).to_broadcast([P, T, HEADS, FIN]),
                in1=c_par["ud"][:].rearrange("p (h f) -> p h f", f=FIN)
                    .unsqueeze(1).to_broadcast([P, T, HEADS, FIN]))
            nc.vector.tensor_reduce(
                out=ad_all[:].rearrange("p (t h) -> p t h", h=HEADS),
                in_=tmp_ad[:].rearrange("p (t h f) -> p t h f", h=HEADS, f=FIN),
                axis=AX, op=OP.add)

            h2_all = res.tile([P, T], F32, tag="h2_all")
            out_all = res.tile([P, T], F32, tag="out_all")

            # ---- quad table in DRAM; fill x columns from input
            xqt = dram.tile([QROWS, TBL_COLS], F32)
            fill = res.tile([P, QROWS * 16 // P], F32, tag="fill")
            nc.sync.dma_start(
                out=fill[:], in_=d_xqx[:].flatten()
                .rearrange("(p j) -> p j", p=P))
            nc.sync.dma_start(out=xqt[:, 0:16], in_=fill[:])

            ad2_all = res.tile([P, T], F32, tag="ad2_all")

            # ============================ layer 1 ============================
            for q in range(NCHUNK):
                K = Kq[q]
                B = CT * K                     # gather blocks in this chunk
                c0, c1 = coloff[q], coloff[q + 1]

                idx_t = io.tile([P, B * P // 16], I16, tag="idx")
                nc.sync.dma_start(out=idx_t[:],
                                  in_=d_idx[:, c0 * 8:c1 * 8])
                sel_t = io.tile([P, B * 4], F32, tag="sel")
                nc.sync.dma_start(out=sel_t[:], in_=d_sel[:, c0 * 4:c1 * 4])
                em_t = io.tile([P, B], F32, tag="em")
                nc.sync.dma_start(out=em_t[:], in_=d_em[:, c0:c1])

                xg = io.tile([P, B * 16], F32, tag="xg")
                xg_r = xg[:].rearrange("p (b e) -> p b e", e=16)
                for b0 in range(0, B, GB):
                    nb = min(GB, B - b0)
                    _dma_gather_small_elem(
                        nc.gpsimd, xg_r[:, b0:b0 + nb, :], d_xqx_gather(xqt),
                        idx_t[:, b0 * 8:(b0 + nb) * 8],
                        num_idxs=nb * P, elem_size=16, elem_step=TBL_COLS)

                # x_eff[p, b, f] = sum_j xg[p, b, 4j+f] * sel[p, b, j]
                xeff = wk.tile([P, B * 4], F32, tag="xeff")
                tsel = wk.tile([P, B * 16], F32, tag="tsel")
                nc.vector.tensor_mul(
                    out=tsel[:].rearrange("p (b f j) -> p b f j", f=4, j=4),
                    in0=xg_r.rearrange("p b (j f) -> p b f j", j=4),
                    in1=sel_t[:].rearrange("p (b j) -> p b j", j=4)
                        .unsqueeze(2).to_broadcast([P, B, 4, 4]))
                nc.vector.tensor_reduce(
                    out=xeff[:].rearrange("p (b f) -> p b f", f=4),
                    in_=tsel[:].rearrange("p (b f j) -> p b f j", f=4, j=4),
                    axis=AX, op=OP.add)

                xeff_r = xeff[:].rearrange("p (t k f) -> p t k f", k=K, f=FIN)
                us_r = c_par["us"][:].rearrange("p (h f) -> p h f", f=FIN)

                # a_s[p, t, h, k] = sum_f xeff[p,t,k,f] * us[h,f]
                e_t = wk.tile([P, CT * HEADS * K], F32, tag="e")
                e_r = e_t[:].rearrange("p (t h k) -> p t h k", h=HEADS, k=K)
                scr = wk.tile([P, CT * HEADS * K], F32, tag="scr")
                scr_r = scr[:].rearrange("p (t h k) -> p t h k", h=HEADS, k=K)
                for f in range(FIN):
                    xf = (xeff_r[:, :, :, f].unsqueeze(2)
                          .to_broadcast([P, CT, HEADS, K]))
                    uf = (us_r[:, :, f].unsqueeze(1).unsqueeze(3)
                          .to_broadcast([P, CT, HEADS, K]))
                    if f == 0:
                        nc.vector.tensor_mul(out=e_r, in0=xf, in1=uf)
                    else:
                        nc.vector.tensor_mul(out=scr_r, in0=xf, in1=uf)
                        nc.vector.tensor_add(out=e_r, in0=e_r, in1=scr_r)

                # e += a_d ; e += emask ; lrelu ; exp
                ad_slice = (ad_all[:].rearrange("p (t h) -> p t h", h=HEADS)
                            [:, q * CT:(q + 1) * CT, :].unsqueeze(3)
                            .to_broadcast([P, CT, HEADS, K]))
                nc.vector.tensor_add(out=e_r, in0=e_r, in1=ad_slice)
                em_r = (em_t[:].rearrange("p (t k) -> p t k", k=K)
                        .unsqueeze(2).to_broadcast([P, CT, HEADS, K]))
                nc.vector.tensor_add(out=e_r, in0=e_r, in1=em_r)
                nc.scalar.activation(out=e_t[:], in_=e_t[:], func=ACT.Prelu,
                                     alpha=NEG_SLOPE)
                nc.scalar.activation(out=e_t[:], in_=e_t[:], func=ACT.Exp)

                # denom & reciprocal
                den = wk.tile([P, CT * HEADS], F32, tag="den")
                nc.vector.tensor_reduce(
                    out=den[:].rearrange("p (t h) -> p t h", h=HEADS),
                    in_=e_r, axis=AX, op=OP.add)
                nc.vector.tensor_scalar(out=den[:], in0=den[:], scalar1=1e-16,
                                        scalar2=None, op0=OP.add)
                rec = wk.tile([P, CT * HEADS], F32, tag="rec")
                nc.vector.reciprocal(out=rec[:], in_=den[:])

                # xw[p, t, h, f] = sum_k e~[p,t,h,k] * xeff[p,t,k,f]
                xw = wk.tile([P, CT * HEADS * FIN], F32, tag="xw")
                xw_r = xw[:].rearrange("p (t h f) -> p t h f", h=HEADS, f=FIN)
                for f in range(FIN):
                    xf = (xeff_r[:, :, :, f].unsqueeze(2)
                          .to_broadcast([P, CT, HEADS, K]))
                    nc.vector.tensor_mul(out=scr_r, in0=e_r, in1=xf)
                    nc.vector.tensor_reduce(out=xw_r[:, :, :, f], in_=scr_r,
                                            axis=AX, op=OP.add)
                nc.vector.tensor_mul(
                    out=xw_r,
                    in0=xw_r,
                    in1=rec[:].rearrange("p (t h) -> p t h", h=HEADS)
                        .unsqueeze(3).to_broadcast([P, CT, HEADS, FIN]))

                # out1[p, t, h, c] = sum_f xw[p,t,h,f] * V[h,c,f]  (+ b1)
                o1 = wk.tile([P, CT * 64], F32, tag="o1")
                o1_r = o1[:].rearrange("p (t h c) -> p t h c", h=HEADS, c=HID)
                t3 = wk.tile([P, CT * HEADS * HID * FIN], F32, tag="t3")
                nc.vector.tensor_mul(
                    out=t3[:].rearrange("p (t h c f) -> p t h c f",
                                        h=HEADS, c=HID, f=FIN),
                    in0=xw_r.unsqueeze(3).to_broadcast([P, CT, HEADS, HID, FIN]),
                    in1=c_par["v3"][:]
                        .rearrange("p (h c f) -> p h c f", c=HID, f=FIN)
                        .unsqueeze(1).to_broadcast([P, CT, HEADS, HID, FIN]))
                nc.vector.tensor_reduce(
                    out=o1_r,
                    in_=t3[:].rearrange("p (t h c f) -> p t h c f",
                                        h=HEADS, c=HID, f=FIN),
                    axis=AX, op=OP.add)
                nc.vector.tensor_add(
                    out=o1[:].rearrange("p (t d) -> p t d", d=64),
                    in0=o1[:].rearrange("p (t d) -> p t d", d=64),
                    in1=c_par["b1v"][:].unsqueeze(1)
                        .to_broadcast([P, CT, 64]))

                # ELU -> h2 = sum_d elu(o1)[d] * W2[d]  (= sum t4*W2 - sum(W2))
                tmin = wk.tile([P, CT * 64], F32, tag="tmin")
                nc.vector.tensor_scalar(out=tmin[:], in0=o1[:], scalar1=0.0,
                                        scalar2=None, op0=OP.min)
                nc.scalar.activation(out=tmin[:], in_=tmin[:], func=ACT.Exp)
                nc.vector.tensor_scalar(out=o1[:], in0=o1[:], scalar1=0.0,
                                        scalar2=None, op0=OP.max)
                nc.vector.tensor_add(out=o1[:], in0=o1[:], in1=tmin[:])
                nc.vector.tensor_mul(
                    out=o1[:].rearrange("p (t d) -> p t d", d=64),
                    in0=o1[:].rearrange("p (t d) -> p t d", d=64),
                    in1=c_par["w2v"][:].unsqueeze(1)
                        .to_broadcast([P, CT, 64]))
                nc.vector.tensor_reduce(
                    out=h2_all[:, q * CT:(q + 1) * CT],
                    in_=o1[:].rearrange("p (t d) -> p t d", d=64),
                    axis=AX, op=OP.add)
                nc.vector.tensor_sub(
                    out=h2_all[:, q * CT:(q + 1) * CT],
                    in0=h2_all[:, q * CT:(q + 1) * CT],
                    in1=c_par["sw2"][:, :1].to_broadcast([P, CT]))

            # ======================= h2 allgather ===========================
            bin_ = dram.tile([MPAD], F32)
            bout = dram.tile([NPAD], F32)
            nc.sync.dma_start(out=bin_[:].rearrange("(t p) -> p t", p=P),
                              in_=h2_all[:])
            nc.gpsimd.collective_compute(
                "AllGather", OP.bypass,
                replica_groups=[list(range(NCORES))],
                ins=[bin_[:]], outs=[bout[:]])
            h2sb = res.tile([P, NPAD // P], F32, tag="h2sb")
            nc.sync.dma_start(out=h2sb[:],
                              in_=bout[:].rearrange("(p j) -> p j", p=P))
            nc.sync.dma_start(out=xqt[:, 16:20], in_=h2sb[:])

            # a_d2 = h2_local * att_dst2
            nc.vector.tensor_mul(
                out=ad2_all[:], in0=h2_all[:],
                in1=c_par["ad2"][:, :1].to_broadcast([P, T]))

            # ============================ layer 2 ============================
            for q in range(NCHUNK):
                K = Kq[q]
                B = CT * K
                c0, c1 = coloff[q], coloff[q + 1]

                idx_t = io.tile([P, B * P // 16], I16, tag="idx")
                nc.sync.dma_start(out=idx_t[:], in_=d_idx[:, c0 * 8:c1 * 8])
                sel_t = io.tile([P, B * 4], F32, tag="sel")
                nc.sync.dma_start(out=sel_t[:], in_=d_sel[:, c0 * 4:c1 * 4])
                em_t = io.tile([P, B], F32, tag="em")
                nc.sync.dma_start(out=em_t[:], in_=d_em[:, c0:c1])

                hg = io.tile([P, B * 4], F32, tag="hg")
                hg_r = hg[:].rearrange("p (b e) -> p b e", e=4)
                for b0 in range(0, B, GB):
                    nb = min(GB, B - b0)
                    _dma_gather_small_elem(
                        nc.gpsimd, hg_r[:, b0:b0 + nb, :], xqt[:, 16:20],
                        idx_t[:, b0 * 8:(b0 + nb) * 8],
                        num_idxs=nb * P, elem_size=4, elem_step=TBL_COLS)

                # h2_eff = sum_j hg[.,j] * sel[.,j]
                hsel = wk.tile([P, B * 4], F32, tag="hsel")
                nc.vector.tensor_mul(out=hsel[:], in0=hg[:], in1=sel_t[:])
                heff = wk.tile([P, B], F32, tag="heff")
                nc.vector.tensor_reduce(
                    out=heff[:],
                    in_=hsel[:].rearrange("p (b j) -> p b j", j=4),
                    axis=AX, op=OP.add)

                e2 = wk.tile([P, B], F32, tag="e2")
                e2_r = e2[:].rearrange("p (t k) -> p t k", k=K)
                nc.vector.tensor_mul(
                    out=e2[:], in0=heff[:],
                    in1=c_par["as2"][:, :1].to_broadcast([P, B]))
                nc.vector.tensor_add(
                    out=e2_r, in0=e2_r,
                    in1=ad2_all[:, q * CT:(q + 1) * CT].unsqueeze(2)
                        .to_broadcast([P, CT, K]))
                nc.vector.tensor_add(out=e2[:], in0=e2[:], in1=em_t[:])
                nc.scalar.activation(out=e2[:], in_=e2[:], func=ACT.Prelu,
                                     alpha=NEG_SLOPE)
                nc.scalar.activation(out=e2[:], in_=e2[:], func=ACT.Exp)

                den2 = wk.tile([P, CT], F32, tag="den2")
                nc.vector.tensor_reduce(out=den2[:], in_=e2_r, axis=AX,
                                        op=OP.add)
                nc.vector.tensor_scalar(out=den2[:], in0=den2[:],
                                        scalar1=1e-16, scalar2=None,
                                        op0=OP.add)
                rec2 = wk.tile([P, CT], F32, tag="rec2")
                nc.vector.reciprocal(out=rec2[:], in_=den2[:])

                num2 = wk.tile([P, B], F32, tag="num2")
                nc.vector.tensor_mul(out=num2[:], in0=e2[:], in1=heff[:])
                o2 = wk.tile([P, CT], F32, tag="o2")
                nc.vector.tensor_reduce(
                    out=o2[:], in_=num2[:].rearrange("p (t k) -> p t k", k=K),
                    axis=AX, op=OP.add)
                nc.vector.tensor_mul(out=o2[:], in0=o2[:], in1=rec2[:])
                nc.vector.tensor_add(
                    out=o2[:], in0=o2[:],
                    in1=c_par["b2v"][:, :1].to_broadcast([P, CT]))
                nc.scalar.activation(out=out_all[:, q * CT:(q + 1) * CT],
                                     in_=o2[:], func=ACT.Sigmoid)

            nc.sync.dma_start(out=d_out[:], in_=out_all[:])

    nc.compile()
    return nc


def d_xqx_gather(xqt):
    return xqt[:, 0:16]


# ------------------------------------------------------------- entry point
_CACHE = {}


def kernel(x, edge_index, W1, att_src1, att_dst1, b1, W2, att_src2, att_dst2,
           b2):
    meta, arrays, order = _prep(x, edge_index)
    params = _fold_params(W1, att_src1, att_dst1, b1, W2, att_src2, att_dst2,
                          b2)

    key = (meta["Kq"], meta["COLS"])
    if key not in _CACHE:
        _CACHE[key] = _build(meta)
    nc = _CACHE[key]

    in_maps = []
    for c in range(NCORES):
        m = {
            "idxg": np.ascontiguousarray(
                arrays["idx_packed"][c].reshape(P, -1)),
            "sel": np.ascontiguousarray(arrays["sel"][c].reshape(P, -1)),
            "emask": np.ascontiguousarray(arrays["emask"][c].reshape(P, -1)),
            "xqx": arrays["xqx"],
            "xl": np.ascontiguousarray(arrays["xl"][c]),
        }
        m.update(params)
        in_maps.append(m)

    res = bass_utils.run_bass_kernel_spmd(nc, in_maps,
                                          core_ids=list(range(NCORES)))

    out = np.empty(N, np.float32)
    for c in range(NCORES):
        vals = res.results[c]["out"].T.ravel()[:M]      # [M] in m-order
        nodes = order[np.arange(M) * NCORES + c]
        out[nodes] = vals
    return out.reshape(N, 1)


# revision 7
# speedup vs baseline: 1.1335x; 1.1335x over previous
"""Trainium2 Bass kernel for nn_GAT_1580547975275 (2-layer GAT, N=100k, E=1.6M).

Strategy (graph/data parallel over 8 NeuronCores, SPMD single program):
- Nodes are ranked by in-degree (host), dealt round-robin to the 8 cores so
  every core sees an identical per-chunk max-degree profile (one shared
  program).  Each core owns M=12500 destination nodes; incoming edges of a
  node occupy K slots of a [128 nodes x K] grid (K = per-chunk max degree).
- Layer-1 message linearity: sum_e alpha_e * h[src_e] = (sum_e alpha_e *
  x[src_e]) @ W1, so per edge we only gather x[src] (16B), not h (256B).
  Attention logits a_s[src] are likewise computed on-device from gathered x
  via folded weights U_s = einsum(W1, att_src1).
- Gathers use the fast SWDGE dma_gather with int16 indices.  Node payloads
  are quad-packed: table row r (256B stride) holds x of gid 4r..4r+3, so row
  indices fit int16 (25088 rows).  A host-built one-hot `sel` mask picks the
  right quarter of each gathered row (and zeroes padding slots).
- Softmax per destination runs over the K axis with an additive -1e9 mask on
  padding slots; the max-subtraction is dropped (mathematically identity).
- h2 (layer-2 scalar feature) is AllGathered across cores inside the same
  NEFF, written into spare columns of the quad table, and layer 2 repeats the
  same gather/softmax with a scalar payload.
"""

import os
import sys

for _p in ("/opt/trn_rl_repo", "/root/.axon_site/_ro/trn_rl_repo"):
    if os.path.isdir(_p) and _p not in sys.path:
        sys.path.insert(0, _p)

import ml_dtypes
import numpy as np

import concourse.bacc as bacc
import concourse.bass as bass
import concourse.mybir as mybir
import concourse.tile as tile
from concourse import ap_utils, bass_utils
from concourse.bass import MemorySpace

# ---------------------------------------------------------------- constants
N = 100000
FIN = 4
HID = 8
HEADS = 8
NEG_SLOPE = 0.2

NCORES = 8
P = 128
M = N // NCORES            # 12500 nodes per core
T = (M + P - 1) // P       # 98 tiles per core
MPAD = T * P               # 12544
NPAD = NCORES * MPAD       # 100352
CT = 7                     # tiles per chunk
NCHUNK = T // CT           # 14
QROWS = NPAD // 4          # 25088 quad rows (int16-safe)
TBL_COLS = 128             # 256B row stride (bf16)
GB = 16                    # gather blocks (x128 idx) per dma_gather (2048 idx, single_packet=False)
NEGBIG = -1.0e9

F32 = mybir.dt.float32
BF16 = mybir.dt.bfloat16
I16 = mybir.dt.int16


# ------------------------------------------------- relaxed dma_gather shim
def _dma_gather_small_elem(eng, out_ap, in_ap, idxs_ap, num_idxs, elem_size,
                           elem_step):
    """nc.gpsimd.dma_gather with the elem_size%256B assert relaxed.

    Vendored from concourse.bass.BassGpSimd.dma_gather (HBM-source,
    non-transpose path).  The 256B-multiple restriction belongs to the
    transpose mode; the ucode's non-transpose path takes elem_size and a
    256B-multiple row stride independently.
    """
    bassmod = sys.modules["concourse.bass"]
    assert idxs_ap.dtype == I16
    assert in_ap.dtype == out_ap.dtype
    elem_bytes = elem_size * mybir.dt.size(in_ap.dtype)
    assert elem_bytes > 0 and elem_bytes % 4 == 0
    assert in_ap.space == MemorySpace.DRAM
    assert idxs_ap.space == MemorySpace.SBUF
    assert out_ap.space == MemorySpace.SBUF
    assert ap_utils.ap_is_contiguous(in_ap.ap[1:])
    assert ap_utils.ap_is_contiguous(out_ap.ap[1:])
    assert ap_utils.ap_is_contiguous(idxs_ap.ap[1:])
    assert in_ap.ap[-1][1] == out_ap.ap[-1][1] == elem_size
    assert out_ap.ap[0][1] * out_ap.ap[1][1] == bassmod.round_up_to_multiple(
        num_idxs, 128)
    assert in_ap.ap[0][0] == elem_step
    stride_bytes = elem_step * mybir.dt.size(in_ap.dtype)
    stride_bytes_256 = bassmod.exact_div(stride_bytes, 256)
    assert stride_bytes_256 < 256

    _in_ap = eng.lower_ap_dma(in_ap, for_custom_bir_dma=True)
    _idxs_ap = eng.lower_ap(idxs_ap)
    _out_ap = eng.lower_ap(out_ap)
    return eng.add_instruction(
        mybir.InstDMAGatherAnt(
            name=eng.bass.get_next_instruction_name(),
            ins=[*_in_ap, _idxs_ap,
                 eng.lower_val_access(eng.to_reg(num_idxs))],
            outs=[_out_ap],
            transpose=False,
            num_idxs=num_idxs,
            elem_size=elem_size,
            stride_bytes_256=stride_bytes_256,
            gen_mode=0,
            single_packet=False,
            queue_num=0,
            sbuf_tokens_per_rank=0,
            sbuf_free_dim_per_rank=0,
            sbuf_free_dim_pad_per_rank=0,
            sbuf_byte_offset=0,
        ))


# ------------------------------------------------------------- host prep
def _prep(x, edge_index):
    src = np.concatenate([np.asarray(edge_index[0]),
                          np.arange(N, dtype=np.int64)])
    dst = np.concatenate([np.asarray(edge_index[1]),
                          np.arange(N, dtype=np.int64)])
    deg = np.bincount(dst, minlength=N)
    order = np.argsort(-deg, kind="stable")
    inv = np.empty(N, np.int64)
    inv[order] = np.arange(N)

    gid = (inv & 7) * MPAD + (inv >> 3)          # node -> gid

    deg_sorted = deg[order]
    Kq = np.empty(NCHUNK, np.int64)
    for q in range(NCHUNK):
        lo = q * CT * P * NCORES
        hi = min((q + 1) * CT * P * NCORES, N)
        Kq[q] = max(1, int(deg_sorted[lo:hi].max())) if lo < N else 1

    cols_q = CT * Kq                              # sel/emask columns per chunk
    coloff = np.concatenate([[0], np.cumsum(cols_q)])
    COLS = int(coloff[-1])                        # per-core grid columns
    TOT = COLS * P                                # per-core padded slots

    # per-edge slot position
    rd = inv[dst]
    cd = rd & 7
    md = rd >> 3
    td = md >> 7
    pd = md & 127
    qd = td // CT
    tq = td % CT
    s_ord = np.argsort(rd, kind="stable")
    rd_s = rd[s_ord]
    new = np.r_[True, rd_s[1:] != rd_s[:-1]]
    starts = np.where(new, np.arange(rd_s.size), 0)
    starts = np.maximum.accumulate(starts)
    k = np.empty(rd_s.size, np.int64)
    k[s_ord] = np.arange(rd_s.size) - starts

    col = coloff[qd] + tq * Kq[qd] + k            # grid column of each edge
    gs = gid[src]

    idxg = np.zeros((NCORES, P, COLS), np.int16)  # quad-row per slot (pad->0)
    sel = np.zeros((NCORES, P, COLS, 4), ml_dtypes.bfloat16)
    emask = np.full((NCORES, P, COLS), NEGBIG, ml_dtypes.bfloat16)
    idxg[cd, pd, col] = (gs >> 2).astype(np.int16)
    sel[cd, pd, col, gs & 3] = 1.0
    emask[cd, pd, col] = 0.0

    # pack idx lists: chunk list order i = (t*Kq+k)*128 + p  ->  [128, L/16]
    # with tile[pp, jj] = list[jj*16 + pp%16]
    idx_packed = np.empty((NCORES, P, TOT // 16), np.int16)
    po16 = 0
    for q in range(NCHUNK):
        L = int(cols_q[q]) * P
        blk = idxg[:, :, coloff[q]:coloff[q + 1]]       # [8, 128, CT*Kq]
        lst = blk.transpose(0, 2, 1)                    # [8, cols, 128] i-major
        lst = lst.reshape(NCORES, L // 16, 16)
        w = lst.transpose(0, 2, 1)                      # [8, 16, L/16]
        idx_packed[:, :, po16:po16 + L // 16] = np.tile(w, (1, 8, 1))
        po16 += L // 16

    # x quad table content (first 16 cols)
    xg16 = np.zeros((NPAD, FIN), np.float32)
    xg16[gid] = np.asarray(x, np.float32)
    xqx = xg16.reshape(QROWS, 16).astype(ml_dtypes.bfloat16)

    # local x per core for a_d
    xl = np.empty((NCORES, P, T * FIN), np.float32)
    for c in range(NCORES):
        xl[c] = (xg16[c * MPAD:(c + 1) * MPAD]
                 .reshape(T, P, FIN).transpose(1, 0, 2).reshape(P, T * FIN))

    meta = dict(Kq=tuple(int(v) for v in Kq), COLS=COLS,
                coloff=tuple(int(v) for v in coloff))
    arrays = dict(idx_packed=idx_packed, sel=sel, emask=emask, xqx=xqx, xl=xl)
    return meta, arrays, order


def _fold_params(W1, att_src1, att_dst1, b1, W2, att_src2, att_dst2, b2):
    W1 = np.asarray(W1, np.float32)
    Wh = W1.reshape(FIN, HEADS, HID)                      # [f, h, c]
    us = np.einsum("fhc,hc->hf", Wh, np.asarray(att_src1, np.float32))
    ud = np.einsum("fhc,hc->hf", Wh, np.asarray(att_dst1, np.float32))
    v3 = Wh.transpose(1, 2, 0).reshape(1, HEADS * HID * FIN)  # [h, c, f]
    rep = lambda a: np.ascontiguousarray(np.tile(np.asarray(a, np.float32)
                                                 .reshape(1, -1), (128, 1)))
    return dict(
        us=rep(us), ud=rep(ud), v3=rep(v3),
        b1v=rep(b1), w2v=rep(W2),
        sw2=rep(np.asarray(W2, np.float32).sum()),
        as2=rep(att_src2), ad2=rep(att_dst2), b2v=rep(b2),
    )


# ---------------------------------------------------------- device program
def _build(meta, ablate=()):
    Kq = meta["Kq"]
    COLS = meta["COLS"]
    coloff = meta["coloff"]
    TOT16 = COLS * P // 16

    nc = bacc.Bacc("TRN2", target_bir_lowering=False, debug=False,
                   num_devices=NCORES, dynamic_dma_scratch_size=65536)
    d_idx = nc.dram_tensor("idxg", [P, TOT16], I16, kind="ExternalInput")
    d_sel = nc.dram_tensor("sel", [P, COLS * 4], BF16, kind="ExternalInput")
    d_em = nc.dram_tensor("emask", [P, COLS], BF16, kind="ExternalInput")
    d_xqx = nc.dram_tensor("xqx", [QROWS, 16], BF16, kind="ExternalInput")
    d_xl = nc.dram_tensor("xl", [P, T * FIN], F32, kind="ExternalInput")
    d_par = {k: nc.dram_tensor(k, [P, n], F32, kind="ExternalInput")
             for k, n in [("us", 32), ("ud", 32), ("v3", 256), ("b1v", 64),
                          ("w2v", 64), ("sw2", 1), ("as2", 1), ("ad2", 1),
                          ("b2v", 1)]}
    d_out = nc.dram_tensor("out", [P, T], F32, kind="ExternalOutput")

    AX = mybir.AxisListType.X
    OP = mybir.AluOpType
    ACT = mybir.ActivationFunctionType

    with tile.TileContext(nc) as tc, \
         nc.allow_low_precision("bf16 4-term selects/logit sums; final accums stay f32"):
        with tc.tile_pool(name="res", bufs=1) as res, \
             tc.tile_pool(name="io", bufs=2) as io, \
             tc.tile_pool(name="wk", bufs=1) as wk, \
             tc.tile_pool(name="dram", bufs=1, space="DRAM") as dram:

            # ---- resident small tensors
            c_par = {}
            for k, d in d_par.items():
                t = res.tile(list(d.shape), F32, tag=f"par_{k}")
                nc.sync.dma_start(out=t[:], in_=d[:])
                c_par[k] = t
            us_bf = res.tile([P, 32], BF16, tag="us_bf")
            nc.vector.tensor_copy(out=us_bf[:], in_=c_par["us"][:])
            xl_t = res.tile([P, T * FIN], F32, tag="xl")
            nc.sync.dma_start(out=xl_t[:], in_=d_xl[:])

            # a_d_all[p, t, h] = sum_f xl[p,t,f] * ud[h,f]
            ad_all = res.tile([P, T * HEADS], F32, tag="ad_all")
            tmp_ad = res.tile([P, T * HEADS * FIN], F32, tag="tmp_ad")
            xl_r = xl_t[:].rearrange("p (t f) -> p t f", f=FIN)
            nc.vector.tensor_mul(
                out=tmp_ad[:].rearrange("p (t h f) -> p t h f", h=HEADS, f=FIN),
                in0=xl_r.unsqueeze(2).to_broadcast([P, T, HEADS, FIN]),
                in1=c_par["ud"][:].rearrange("p (h f) -> p h f", f=FIN)
                    .unsqueeze(1).to_broadcast([P, T, HEADS, FIN]))
            nc.vector.tensor_reduce(
                out=ad_all[:].rearrange("p (t h) -> p t h", h=HEADS),
                in_=tmp_ad[:].rearrange("p (t h f) -> p t h f", h=HEADS, f=FIN),
                axis=AX, op=OP.add)

            ad_bf = res.tile([P, T * HEADS], BF16, tag="ad_bf")
            nc.vector.tensor_copy(out=ad_bf[:], in_=ad_all[:])
            h2_all = res.tile([P, T], F32, tag="h2_all")
            out_all = res.tile([P, T], F32, tag="out_all")

            # ---- quad table in DRAM; fill x columns from input
            xqt = dram.tile([QROWS, TBL_COLS], BF16)
            fill = res.tile([P, QROWS * 16 // P], BF16, tag="fill")
            nc.sync.dma_start(
                out=fill[:], in_=d_xqx[:].flatten()
                .rearrange("(p j) -> p j", p=P))
            nc.sync.dma_start(out=xqt[:, 0:16], in_=fill[:])

            ad2_all = res.tile([P, T], F32, tag="ad2_all")

            # ============================ layer 1 ============================
            for q in range(NCHUNK):
                K = Kq[q]
                B = CT * K                     # gather blocks in this chunk
                c0, c1 = coloff[q], coloff[q + 1]

                idx_t = io.tile([P, B * P // 16], I16, tag="idx")
                nc.sync.dma_start(out=idx_t[:],
                                  in_=d_idx[:, c0 * 8:c1 * 8])
                sel_t = io.tile([P, B * 4], BF16, tag="sel")
                nc.sync.dma_start(out=sel_t[:], in_=d_sel[:, c0 * 4:c1 * 4])
                em_t = io.tile([P, B], BF16, tag="em")
                nc.sync.dma_start(out=em_t[:], in_=d_em[:, c0:c1])

                xg = io.tile([P, B * 16], BF16, tag="xg")
                xg_r = xg[:].rearrange("p (b e) -> p b e", e=16)
                if "l1gather" in ablate:
                    nc.vector.memset(xg[:], 0.0)
                for b0 in ([] if "l1gather" in ablate else range(0, B, GB)):
                    nb = min(GB, B - b0)
                    _dma_gather_small_elem(
                        nc.gpsimd, xg_r[:, b0:b0 + nb, :], d_xqx_gather(xqt),
                        idx_t[:, b0 * 8:(b0 + nb) * 8],
                        num_idxs=nb * P, elem_size=16, elem_step=TBL_COLS)

                # x_eff[p, b, f] = sum_j xg[p, b, 4j+f] * sel[p, b, j]
                xeff = wk.tile([P, B * 4], BF16, tag="xeff")
                Bh = (B + 1) // 2
                tsel = wk.tile([P, Bh * 16], BF16, tag="tsel")
                for h0, h1 in ((0, Bh), (Bh, B)):
                    n = h1 - h0
                    nc.vector.tensor_mul(
                        out=tsel[:, :n * 16]
                            .rearrange("p (b f j) -> p b f j", f=4, j=4),
                        in0=xg_r[:, h0:h1, :]
                            .rearrange("p b (j f) -> p b f j", j=4),
                        in1=sel_t[:].rearrange("p (b j) -> p b j", j=4)
                            [:, h0:h1, :].unsqueeze(2)
                            .to_broadcast([P, n, 4, 4]))
                    nc.vector.tensor_reduce(
                        out=xeff[:, h0 * 4:h1 * 4]
                            .rearrange("p (b f) -> p b f", f=4),
                        in_=tsel[:, :n * 16]
                            .rearrange("p (b f j) -> p b f j", f=4, j=4),
                        axis=AX, op=OP.add)

                xeff_r = xeff[:].rearrange("p (t k f) -> p t k f", k=K, f=FIN)
                us_r = us_bf[:].rearrange("p (h f) -> p h f", f=FIN)

                # a_s[p, t, h, k] = sum_f xeff[p,t,k,f] * us[h,f]
                e_t = wk.tile([P, CT * HEADS * K], BF16, tag="e")
                e_r = e_t[:].rearrange("p (t h k) -> p t h k", h=HEADS, k=K)
                scr = wk.tile([P, CT * HEADS * K], BF16, tag="scr")
                scr_r = scr[:].rearrange("p (t h k) -> p t h k", h=HEADS, k=K)
                for f in range(FIN):
                    xf = (xeff_r[:, :, :, f].unsqueeze(2)
                          .to_broadcast([P, CT, HEADS, K]))
                    uf = (us_r[:, :, f].unsqueeze(1).unsqueeze(3)
                          .to_broadcast([P, CT, HEADS, K]))
                    if f == 0:
                        nc.vector.tensor_mul(out=e_r, in0=xf, in1=uf)
                    else:
                        nc.vector.tensor_mul(out=scr_r, in0=xf, in1=uf)
                        nc.vector.tensor_add(out=e_r, in0=e_r, in1=scr_r)

                # e += a_d ; e += emask ; lrelu ; exp
                ad_slice = (ad_bf[:].rearrange("p (t h) -> p t h", h=HEADS)
                            [:, q * CT:(q + 1) * CT, :].unsqueeze(3)
                            .to_broadcast([P, CT, HEADS, K]))
                nc.vector.tensor_add(out=e_r, in0=e_r, in1=ad_slice)
                em_r = (em_t[:].rearrange("p (t k) -> p t k", k=K)
                        .unsqueeze(2).to_broadcast([P, CT, HEADS, K]))
                nc.vector.tensor_add(out=e_r, in0=e_r, in1=em_r)
                nc.scalar.activation(out=e_t[:], in_=e_t[:], func=ACT.Prelu,
                                     alpha=NEG_SLOPE)
                nc.scalar.activation(out=e_t[:], in_=e_t[:], func=ACT.Exp)

                # denom & reciprocal
                den = wk.tile([P, CT * HEADS], F32, tag="den")
                nc.vector.tensor_reduce(
                    out=den[:].rearrange("p (t h) -> p t h", h=HEADS),
                    in_=e_r, axis=AX, op=OP.add)
                nc.vector.tensor_scalar(out=den[:], in0=den[:], scalar1=1e-16,
                                        scalar2=None, op0=OP.add)
                rec = wk.tile([P, CT * HEADS], F32, tag="rec")
                nc.vector.reciprocal(out=rec[:], in_=den[:])

                # xw[p, t, h, f] = sum_k e~[p,t,h,k] * xeff[p,t,k,f]
                xw = wk.tile([P, CT * HEADS * FIN], F32, tag="xw")
                xw_r = xw[:].rearrange("p (t h f) -> p t h f", h=HEADS, f=FIN)
                for f in range(FIN):
                    xf = (xeff_r[:, :, :, f].unsqueeze(2)
                          .to_broadcast([P, CT, HEADS, K]))
                    nc.vector.tensor_mul(out=scr_r, in0=e_r, in1=xf)
                    nc.vector.tensor_reduce(out=xw_r[:, :, :, f], in_=scr_r,
                                            axis=AX, op=OP.add)
                nc.vector.tensor_mul(
                    out=xw_r,
                    in0=xw_r,
                    in1=rec[:].rearrange("p (t h) -> p t h", h=HEADS)
                        .unsqueeze(3).to_broadcast([P, CT, HEADS, FIN]))

                # out1[p, t, h, c] = sum_f xw[p,t,h,f] * V[h,c,f]  (+ b1)
                o1 = wk.tile([P, CT * 64], F32, tag="o1")
                o1_r = o1[:].rearrange("p (t h c) -> p t h c", h=HEADS, c=HID)
                t3 = wk.tile([P, CT * HEADS * HID * FIN], F32, tag="t3")
                nc.vector.tensor_mul(
                    out=t3[:].rearrange("p (t h c f) -> p t h c f",
                                        h=HEADS, c=HID, f=FIN),
                    in0=xw_r.unsqueeze(3).to_broadcast([P, CT, HEADS, HID, FIN]),
                    in1=c_par["v3"][:]
                        .rearrange("p (h c f) -> p h c f", c=HID, f=FIN)
                        .unsqueeze(1).to_broadcast([P, CT, HEADS, HID, FIN]))
                nc.vector.tensor_reduce(
                    out=o1_r,
                    in_=t3[:].rearrange("p (t h c f) -> p t h c f",
                                        h=HEADS, c=HID, f=FIN),
                    axis=AX, op=OP.add)
                nc.vector.tensor_add(
                    out=o1[:].rearrange("p (t d) -> p t d", d=64),
                    in0=o1[:].rearrange("p (t d) -> p t d", d=64),
                    in1=c_par["b1v"][:].unsqueeze(1)
                        .to_broadcast([P, CT, 64]))

                # ELU -> h2 = sum_d elu(o1)[d] * W2[d]  (= sum t4*W2 - sum(W2))
                tmin = wk.tile([P, CT * 64], F32, tag="tmin")
                nc.vector.tensor_scalar(out=tmin[:], in0=o1[:], scalar1=0.0,
                                        scalar2=None, op0=OP.min)
                nc.scalar.activation(out=tmin[:], in_=tmin[:], func=ACT.Exp)
                nc.vector.tensor_scalar(out=o1[:], in0=o1[:], scalar1=0.0,
                                        scalar2=None, op0=OP.max)
                nc.vector.tensor_add(out=o1[:], in0=o1[:], in1=tmin[:])
                nc.vector.tensor_mul(
                    out=o1[:].rearrange("p (t d) -> p t d", d=64),
                    in0=o1[:].rearrange("p (t d) -> p t d", d=64),
                    in1=c_par["w2v"][:].unsqueeze(1)
                        .to_broadcast([P, CT, 64]))
                nc.vector.tensor_reduce(
                    out=h2_all[:, q * CT:(q + 1) * CT],
                    in_=o1[:].rearrange("p (t d) -> p t d", d=64),
                    axis=AX, op=OP.add)
                nc.vector.tensor_sub(
                    out=h2_all[:, q * CT:(q + 1) * CT],
                    in0=h2_all[:, q * CT:(q + 1) * CT],
                    in1=c_par["sw2"][:, :1].to_broadcast([P, CT]))

            # ======================= h2 allgather ===========================
            bin_ = dram.tile([MPAD], F32)
            bout = dram.tile([NPAD], F32)
            nc.sync.dma_start(out=bin_[:].rearrange("(t p) -> p t", p=P),
                              in_=h2_all[:])
            nc.gpsimd.collective_compute(
                "AllGather", OP.bypass,
                replica_groups=[list(range(NCORES))],
                ins=[bin_[:]], outs=[bout[:]])
            h2sb = res.tile([P, NPAD // P], F32, tag="fill")
            nc.sync.dma_start(out=h2sb[:],
                              in_=bout[:].rearrange("(p j) -> p j", p=P))
            nc.gpsimd.dma_start(out=xqt[:, 16:20], in_=h2sb[:])

            # a_d2 = h2_local * att_dst2
            nc.vector.tensor_mul(
                out=ad2_all[:], in0=h2_all[:],
                in1=c_par["ad2"][:, :1].to_broadcast([P, T]))

            # ============================ layer 2 ============================
            for q in range(NCHUNK):
                K = Kq[q]
                B = CT * K
                c0, c1 = coloff[q], coloff[q + 1]

                idx_t = io.tile([P, B * P // 16], I16, tag="idx")
                nc.sync.dma_start(out=idx_t[:], in_=d_idx[:, c0 * 8:c1 * 8])
                sel_t = io.tile([P, B * 4], BF16, tag="sel")
                nc.sync.dma_start(out=sel_t[:], in_=d_sel[:, c0 * 4:c1 * 4])
                em_t = io.tile([P, B], BF16, tag="em")
                nc.sync.dma_start(out=em_t[:], in_=d_em[:, c0:c1])

                hg = io.tile([P, B * 4], BF16, tag="hg")
                hg_r = hg[:].rearrange("p (b e) -> p b e", e=4)
                if "l2gather" in ablate:
                    nc.vector.memset(hg[:], 0.0)
                for b0 in ([] if "l2gather" in ablate else range(0, B, GB)):
                    nb = min(GB, B - b0)
                    _dma_gather_small_elem(
                        nc.gpsimd, hg_r[:, b0:b0 + nb, :], xqt[:, 16:20],
                        idx_t[:, b0 * 8:(b0 + nb) * 8],
                        num_idxs=nb * P, elem_size=4, elem_step=TBL_COLS)

                # h2_eff = sum_j hg[.,j] * sel[.,j]
                hsel = wk.tile([P, B * 4], F32, tag="hsel")
                nc.vector.tensor_mul(out=hsel[:], in0=hg[:], in1=sel_t[:])
                heff = wk.tile([P, B], F32, tag="heff")
                nc.vector.tensor_reduce(
                    out=heff[:],
                    in_=hsel[:].rearrange("p (b j) -> p b j", j=4),
                    axis=AX, op=OP.add)

                e2 = wk.tile([P, B], F32, tag="e2")
                e2_r = e2[:].rearrange("p (t k) -> p t k", k=K)
                nc.vector.tensor_mul(
                    out=e2[:], in0=heff[:],
                    in1=c_par["as2"][:, :1].to_broadcast([P, B]))
                nc.vector.tensor_add(
                    out=e2_r, in0=e2_r,
                    in1=ad2_all[:, q * CT:(q + 1) * CT].unsqueeze(2)
                        .to_broadcast([P, CT, K]))
                nc.vector.tensor_add(out=e2[:], in0=e2[:], in1=em_t[:])
                nc.scalar.activation(out=e2[:], in_=e2[:], func=ACT.Prelu,
                                     alpha=NEG_SLOPE)
                nc.scalar.activation(out=e2[:], in_=e2[:], func=ACT.Exp)

                den2 = wk.tile([P, CT], F32, tag="den2")
                nc.vector.tensor_reduce(out=den2[:], in_=e2_r, axis=AX,
                                        op=OP.add)
                nc.vector.tensor_scalar(out=den2[:], in0=den2[:],
                                        scalar1=1e-16, scalar2=None,
                                        op0=OP.add)
                rec2 = wk.tile([P, CT], F32, tag="rec2")
                nc.vector.reciprocal(out=rec2[:], in_=den2[:])

                num2 = wk.tile([P, B], F32, tag="num2")
                nc.vector.tensor_mul(out=num2[:], in0=e2[:], in1=heff[:])
                o2 = wk.tile([P, CT], F32, tag="o2")
                nc.vector.tensor_reduce(
                    out=o2[:], in_=num2[:].rearrange("p (t k) -> p t k", k=K),
                    axis=AX, op=OP.add)
                nc.vector.tensor_mul(out=o2[:], in0=o2[:], in1=rec2[:])
                nc.vector.tensor_add(
                    out=o2[:], in0=o2[:],
                    in1=c_par["b2v"][:, :1].to_broadcast([P, CT]))
                nc.scalar.activation(out=out_all[:, q * CT:(q + 1) * CT],
                                     in_=o2[:], func=ACT.Sigmoid)

            nc.sync.dma_start(out=d_out[:], in_=out_all[:])

    nc.compile()
    return nc


def d_xqx_gather(xqt):
    return xqt[:, 0:16]


# ------------------------------------------------------------- entry point
_CACHE = {}


def kernel(x, edge_index, W1, att_src1, att_dst1, b1, W2, att_src2, att_dst2,
           b2):
    meta, arrays, order = _prep(x, edge_index)
    params = _fold_params(W1, att_src1, att_dst1, b1, W2, att_src2, att_dst2,
                          b2)

    key = (meta["Kq"], meta["COLS"])
    if key not in _CACHE:
        _CACHE[key] = _build(meta)
    nc = _CACHE[key]

    in_maps = []
    for c in range(NCORES):
        m = {
            "idxg": np.ascontiguousarray(
                arrays["idx_packed"][c].reshape(P, -1)),
            "sel": np.ascontiguousarray(arrays["sel"][c].reshape(P, -1)),
            "emask": np.ascontiguousarray(arrays["emask"][c].reshape(P, -1)),
            "xqx": arrays["xqx"],
            "xl": np.ascontiguousarray(arrays["xl"][c]),
        }
        m.update(params)
        in_maps.append(m)

    res = bass_utils.run_bass_kernel_spmd(nc, in_maps,
                                          core_ids=list(range(NCORES)))

    out = np.empty(N, np.float32)
    for c in range(NCORES):
        vals = res.results[c]["out"].T.ravel()[:M]      # [M] in m-order
        nodes = order[np.arange(M) * NCORES + c]
        out[nodes] = vals
    return out.reshape(N, 1)
